# revision 1
# baseline (speedup 1.0000x reference)
"""Trainium2 Bass kernel for DocumentBertScoringLoss (B=8192).

loss = MSE(p, g) + MR(p, g) + SIM(p, g), returned as shape-(1,) fp32.

Key identity (verified numerically): summing the margin-ranking hinge over
all ordered pairs (m, n), with r = sign(dp) (or -sign(dg) at ties, which
does not matter because r*dp = 0 there),

    sum max(0, 0.1 - r*dp) = 0.1*B^2 - 2 * sum clamp(p_m - p_n, 0, 0.1)

so the whole BxB hinge reduces to one clamp per pair.  Per device (row
stripe of 1024 rows), partition p / chunk c holds row value s1 = p_i and
the full prediction vector is broadcast along the free dim (X, fp16).
One chained DVE tensor_scalar computes h = min(max(X, s1), s1 + 0.1)
= s1 + clamp(p_n - p_m, 0, 0.1) (by (m,n) symmetry of the full double
sum the sign of the difference does not matter).  h tiles are summed by
the PE (ones-matmul accumulated in PSUM) and by the scalar engine
(Identity activation with accum_out); 8192*s1 is subtracted at the end.

Sharding: rows of the pairwise matrix, 1024 per core; predictions /
correct_output replicated.  Each core outputs its additive contribution
c_k; the host gather is a plain sum of the 8 scalars (the "all-reduce").
"""

import numpy as np

import concourse.bass as bass
import concourse.bacc as bacc
import concourse.mybir as mybir
from concourse.bass_utils import run_bass_kernel_spmd
from concourse.tile import TileContext
from concourse.alu_op_type import AluOpType

B = 8192
NCORES = 8
ROWS_PER_CORE = B // NCORES          # 1024
NCHUNK = ROWS_PER_CORE // 128        # 8 row chunks of 128 partitions
HALF = 4096                          # column tile width for the main pass
NHALF = B // HALF                    # 2
MR_BIAS = 0.1

# Column split inside each half-tile: [0, C_PE) reduced on the PE via
# ones-matmul, [C_PE, HALF) reduced on the scalar engine via accum_out.
C_PE = 2944
N_WARM = 8
MM_N = 512                           # PSUM bank limit for fp32 out

F32 = mybir.dt.float32
F16 = mybir.dt.float16

_CACHED = {}


def _tt(nc, out, in0, in1, op):
    # tensor_tensor via the TensorScalarPtr ISA struct ((0 + in0) op in1):
    # the gen3 TensorTensor struct only carries one sync-wait slot, which
    # the Tile scheduler can exceed; TSP carries more.
    nc.vector.scalar_tensor_tensor(out, in0, 0.0, in1, AluOpType.add, op)



def _build_nc():
    nc = bacc.Bacc("TRN2", target_bir_lowering=False, debug=False, num_devices=NCORES)

    pred_d = nc.dram_tensor("predictions", [B], F32, kind="ExternalInput")
    g_d = nc.dram_tensor("correct_output", [B], F32, kind="ExternalInput")
    # p_rows arrives transposed [8, 128] (prow_t[c, p] = row value of
    # chunk c / partition p): an [8, 128] DMA is 8 fat descriptors
    # (~0.8us) instead of the 128 tiny ones a direct [128, 8] fill needs
    # (~3.3us, which gated the first clamp).  A tiny PE matmul against an
    # 8x8 identity transposes it on chip.
    prow_d = nc.dram_tensor("p_rows", [NCHUNK, 128], F32, kind="ExternalInput")
    eye_d = nc.dram_tensor("eye8", [NCHUNK, NCHUNK], F32, kind="ExternalInput")
    out_d = nc.dram_tensor("out", [2], F32, kind="ExternalOutput")

    AF = mybir.ActivationFunctionType

    with TileContext(nc) as tc:
        with (
            tc.tile_pool(name="const", bufs=1) as cpool,
            tc.tile_pool(name="hbuf", bufs=3) as hpool,
            tc.tile_pool(name="psum", bufs=1, space="PSUM") as ppool,
        ):
            # ---- persistent tiles ----
            xbf = cpool.tile([128, B], F16, name="xbf")
            pred32 = cpool.tile([128, B // 128], F32, name="pred32")
            g32 = cpool.tile([128, B // 128], F32, name="g32")
            prow = cpool.tile([128, NCHUNK], F32, name="prow")
            s2 = cpool.tile([128, NCHUNK], F32, name="s2")
            ones_bf = cpool.tile([128, 1], F16, name="ones_bf")
            ones_f32 = cpool.tile([128, 1], F32, name="ones_f32")
            stacked = cpool.tile([128, 6], F32, name="stacked")
            d_tile = cpool.tile([128, B // 128], F32, name="d_tile")
            junk_sq = cpool.tile([128, B // 128], F32, name="junk_sq")
            junk_stt = cpool.tile([128, B // 128], F32, name="junk_stt")
            sc = cpool.tile([1, 16], F32, name="sc")
            out_sb2 = cpool.tile([1, 2], F32, name="out_sb2")

            psum_main = ppool.tile([128, MM_N], F32, name="psum_main")
            psum_warm = ppool.tile([128, MM_N], F32, name="psum_warm")
            psum_small = ppool.tile([128, 8], F32, name="psum_small")
            psum_acc = ppool.tile([128, NCHUNK * NHALF], F32, name="psum_acc")

            # ---- input DMAs ----
            pred_ap = pred_d[:]
            prow_t = cpool.tile([NCHUNK, 128], F32, name="prow_t")
            nc.sync.dma_start(prow_t, prow_d[:, :])
            eye8 = cpool.tile([NCHUNK, NCHUNK], F32, name="eye8")
            nc.sync.dma_start(eye8, eye_d[:, :])
            # pred32/g32 trigger from the scalar engine's HWDGE queue so the
            # tiny prow_t/eye8 transfers (which gate the first clamp) are
            # not queued behind them.
            nc.scalar.dma_start(pred32, pred_ap.rearrange("(p c) -> p c", p=128))
            nc.scalar.dma_start(g32, g_d[:].rearrange("(p c) -> p c", p=128))

            # Broadcast predictions along partitions into X [128, B] fp16.
            # Column blocks pipeline the transfer so the first clamp starts
            # early.  Blocks 0-1 cast-broadcast straight from the f32 input
            # (lowest latency); the rest broadcast from a 16KB fp16 DRAM
            # scratch written on-chip, which halves the HBM read volume of
            # the 128x re-read (all 8 cores broadcast simultaneously, so
    
            # HBM pressure is the multi-core risk).
            XBLK = 1024
            NDIRECT = 4
            scratch16 = nc.dram_tensor("pred16_scratch", [B], F16, kind="Internal")
            for j in range(NDIRECT):
                nc.gpsimd.dma_start(
                    xbf[:, j * XBLK:(j + 1) * XBLK],
                    pred_ap[j * XBLK:(j + 1) * XBLK].partition_broadcast(128),
                )
            # DRAM->DRAM cast (f32 -> fp16) with no SBUF roundtrip and no
            # upstream dependency, queued after the latency-critical direct
            # blocks so the Q7 descriptor queue never head-of-line blocks.
            nc.gpsimd.dma_start(scratch16[:], pred_ap)
            for j in range(NDIRECT, B // XBLK):
                nc.gpsimd.dma_start(
                    xbf[:, j * XBLK:(j + 1) * XBLK],
                    scratch16[j * XBLK:(j + 1) * XBLK].partition_broadcast(128),
                )

            zeros1 = cpool.tile([128, 1], F32, name="zeros1")
            nc.vector.memset(zeros1, 0.0)

            # on-chip transpose of prow_t [8,128] -> prow [128,8] via a
            # matmul against a host-provided 8x8 identity (engines cannot
            # write single non-32-aligned partitions to build it on chip).
            psum_pr = ppool.tile([128, NCHUNK], F32, name="psum_pr")
            nc.tensor.matmul(psum_pr, prow_t, eye8, start=True, stop=True)
            nc.vector.tensor_copy(prow, psum_pr)

            # Dummy sqrt issued first so the single act-table load picks a
            # set containing sqrt+identity+square (avoids a second ~1.3us
            # LoadActFuncSet right before the final scalar chain).
            warm_sqrt = cpool.tile([1, 1], F32, name="warm_sqrt")
            nc.scalar.activation(warm_sqrt, zeros1[0:1, :], AF.Sqrt,
                                 bias=zeros1[0:1, :])
            nc.vector.memset(ones_bf, 1.0)
            nc.vector.memset(ones_f32, 1.0)
            nc.vector.tensor_scalar(s2, prow, MR_BIAS, None, AluOpType.add)


            # PE warm-up: dense dummy matmuls from t~0.5us keep the HAM
            # activity window busy so the real matmul stream runs at the
            # warm clock from its first instruction.
            junk_bf = cpool.tile([128, MM_N], F16, name="junk_bf")
            nc.vector.memset(junk_bf, 0.0)
            for _w in range(N_WARM):
                nc.tensor.matmul(
                    psum_warm[0:1, 0:MM_N], ones_bf, junk_bf,
                    start=True, stop=True,
                )

            # ---- main pass: h = min(max(X, s1), s1 + 0.1) ----
            C_ACT = HALF - C_PE
            # main-MM count: 14 normal half-chunks x ceil(C_PE/512) + chunk 6
            # (full width, 8 MMs); chunk 7 reduces on the DVE instead.
            mm_total = 14 * ((C_PE + MM_N - 1) // MM_N) + HALF // MM_N
            mm_idx = 0
            acc_slots = []
            # half-outer loop: all chunks of column-half 0 run while the
            # second half of the broadcast is still in flight.
            for hh in range(NHALF):
                x_half = xbf[:, hh * HALF:(hh + 1) * HALF]
                for cp in range(NCHUNK // 2):
                    last_pair = (hh == NHALF - 1 and cp == NCHUNK // 2 - 1)
                    # ACT tile shared by two consecutive chunks: halves the
                    # per-instruction overhead of the ACT accumulation.
                    h_act = None
                    if not last_pair:
                        h_act = hpool.tile(
                            [128, 2 * C_ACT], F16, tag="h_act", name="h_act",
                            bufs=4,
                        )
                    c_pe_here = HALF if last_pair else C_PE
                    for ci in range(2):
                        c = 2 * cp + ci
                        # The first chunks use narrower clamp pieces so the
                        # PE starts as soon as the first broadcast blocks
                        # land, instead of waiting for the full half.
                        if hh == 0 and cp == 0 and ci == 0:
                            cuts = [0, 1024, 2048, C_PE]
                        elif False:
                            cuts = [0, 2048, C_PE]
                        else:
                            cuts = [0, c_pe_here]
                        for p0, p1 in zip(cuts, cuts[1:]):
                            h_pe = hpool.tile(
                                [128, p1 - p0], F16, tag="h_pe", name="h_pe",
                                bufs=5,
                            )
                            nc.vector.tensor_scalar(
                                h_pe[:, 0:p1 - p0],
                                x_half[:, p0:p1],
                                prow[:, c:c + 1],
                                s2[:, c:c + 1],
                                AluOpType.max,
                                AluOpType.min,
                            )
                            if last_pair and ci == 1:
                                # very last chunk: reduce on the DVE itself
                                # (tensor_scalar add with accum) so the tail
                                # skips both the PE matmuls and the 658ns
                                # single-lane PSUM reduce
                                a_slot = hpool.tile(
                                    [128, 1], F32, tag="a_slot",
                                    bufs=NCHUNK, name="a_slot",
                                )
                                acc_slots.append(a_slot)
                                nc.vector.tensor_scalar(
                                    h_pe, h_pe, 0.0, None,
                                    AluOpType.add, AluOpType.add,
                                    accum_out=a_slot,
                                )
                                continue
                            for n0 in range(p0, p1, MM_N):
                                n1 = min(n0 + MM_N, p1)
                                nc.tensor.matmul(
                                    psum_main[0:1, 0:n1 - n0],
                                    ones_bf,
                                    h_pe[:, n0 - p0:n1 - p0],
                                    start=(mm_idx == 0),
                                    stop=(mm_idx == mm_total - 1),
                                    skip_group_check=True,
                                )
                                mm_idx += 1
                        if not last_pair:
                            nc.vector.tensor_scalar(
                                h_act[:, ci * C_ACT:(ci + 1) * C_ACT],
                                x_half[:, C_PE:HALF],
                                prow[:, c:c + 1],
                                s2[:, c:c + 1],
                                AluOpType.max,
                                AluOpType.min,
                            )
                    if not last_pair:
                        a_slot = hpool.tile([128, 1], F32, tag="a_slot",
                                            bufs=NCHUNK, name="a_slot")
                        acc_slots.append(a_slot)
                        nc.scalar.activation(
                            h_act,
                            h_act,
                            AF.Identity,
                            bias=zeros1,
                            accum_out=a_slot,
                        )

            # ---- small terms ----
            # Sum the per-iteration ACT accumulators on the PE: one matmul
            # per slot accumulating into one PSUM scalar.
            for si_, a_slot in enumerate(acc_slots):
                nc.tensor.matmul(
                    psum_acc[0:1, 0:1],
                    ones_f32,
                    a_slot,
                    start=(si_ == 0),
                    stop=(si_ == len(acc_slots) - 1),
                )
            nc.vector.tensor_reduce(
                stacked[:, 1:2], prow, mybir.AxisListType.X, AluOpType.add
            )
            _tt(nc, d_tile, pred32, g32, AluOpType.subtract)
            sq_acc = cpool.tile([128, 1], F32, name="sq_acc")
            nc.scalar.activation(
                junk_sq, d_tile, AF.Square, bias=zeros1, accum_out=sq_acc
            )
            nc.vector.tensor_copy(stacked[:, 2:3], sq_acc)
            nc.vector.scalar_tensor_tensor(
                junk_stt, pred32, 1.0, g32, AluOpType.mult, AluOpType.mult,
                accum_out=stacked[:, 3:4],
            )
            nc.vector.scalar_tensor_tensor(
                junk_stt, pred32, 1.0, pred32, AluOpType.mult, AluOpType.mult,
                accum_out=stacked[:, 4:5],
            )
            nc.vector.scalar_tensor_tensor(
                junk_stt, g32, 1.0, g32, AluOpType.mult, AluOpType.mult,
                accum_out=stacked[:, 5:6],
            )

            # partition reduction: [1, 6] = ones^T @ stacked
            nc.tensor.matmul(
                psum_small[0:1, 0:6], ones_f32, stacked, start=True, stop=True
            )

            # ---- final scalar assembly (partition 0) ----
            smalls = cpool.tile([1, 6], F32, name="smalls")
            nc.vector.tensor_copy(smalls, psum_small[0:1, 0:6])
            t_act = sc[0:1, 13:14]
            nc.vector.tensor_copy(t_act, psum_acc[0:1, 0:1])
            p_sum = smalls[0:1, 1:2]
            sq = smalls[0:1, 2:3]
            dot = smalls[0:1, 3:4]
            pp = smalls[0:1, 4:5]
            gg = smalls[0:1, 5:6]

            tpe = sc[0:1, 0:1]
            nc.vector.tensor_reduce(
                tpe, psum_main[0:1, 0:MM_N], mybir.AxisListType.X, AluOpType.add
            )
            corr = sc[0:1, 2:3]
            nc.vector.tensor_scalar(corr, p_sum, float(B), None, AluOpType.mult)
            # v = k*(t_act - corr) computes while the tpe reduce runs; the
            # tail then needs a single op after tpe.
            K2 = -2.0 / (float(B) * float(B))
            v1 = sc[0:1, 1:2]
            nc.vector.scalar_tensor_tensor(
                v1, t_act, 1.0, corr, AluOpType.mult, AluOpType.subtract
            )
            v2 = sc[0:1, 3:4]
            nc.vector.tensor_scalar(v2, v1, K2, None, AluOpType.mult)

            mse_part = sc[0:1, 5:6]
            nc.vector.tensor_scalar(
                mse_part, sq, 1.0 / (float(B) * NCORES), None, AluOpType.mult
            )
            prod = sc[0:1, 6:7]
            _tt(nc, prod, pp, gg, AluOpType.mult)
            denom = sc[0:1, 7:8]
            nc.scalar.activation(denom, prod, AF.Sqrt, bias=zeros1[0:1, :])
            dmax = sc[0:1, 8:9]
            nc.vector.tensor_scalar(dmax, denom, 1e-8, None, AluOpType.max)
            inv = sc[0:1, 9:10]
            nc.vector.reciprocal(inv, dmax)
            sims = sc[0:1, 10:11]
            _tt(nc, sims, dot, inv, AluOpType.mult)
            # sim_part = (1 - sims)/8
            sim_part = sc[0:1, 11:12]
            nc.vector.tensor_scalar(
                sim_part, sims, -1.0 / NCORES, 1.0 / NCORES,
                AluOpType.mult, AluOpType.add,
            )
            # out[0] = mse/8 + (1-sim)/8 + 0.1/8 completes early (only
            # psum_small-dependent); out[1] = -2*S/B^2 is the tail-critical
            # value.  The host sum over 16 numbers is unchanged math.
            early = sc[0:1, 12:13]
            _tt(nc, early, mse_part, sim_part, AluOpType.add)
            nc.vector.tensor_scalar(
                out_sb2[0:1, 0:1], early, MR_BIAS / NCORES, None, AluOpType.add
            )
            nc.vector.scalar_tensor_tensor(
                out_sb2[0:1, 1:2], tpe, K2, v2, AluOpType.mult, AluOpType.add
            )
            nc.sync.dma_start(out_d[None, :], out_sb2)

    nc.compile()
    return nc


def kernel(predictions: np.ndarray, correct_output: np.ndarray) -> np.ndarray:
    pred = np.ascontiguousarray(np.asarray(predictions, dtype=np.float32))
    g = np.ascontiguousarray(np.asarray(correct_output, dtype=np.float32))

    if "nc" not in _CACHED:
        _CACHED["nc"] = _build_nc()
    nc = _CACHED["nc"]

    in_maps = []
    for k in range(NCORES):
        in_maps.append(
            {
                "predictions": pred,
                "correct_output": g,
                "p_rows": np.ascontiguousarray(
                    pred[k * ROWS_PER_CORE:(k + 1) * ROWS_PER_CORE]
                    .reshape(128, NCHUNK).T
                ),
                "eye8": np.eye(NCHUNK, dtype=np.float32),
            }
        )

    res = None
    last_exc = None
    for _attempt in range(3):
        try:
            res = run_bass_kernel_spmd(nc, in_maps, core_ids=list(range(NCORES)))
            break
        except Exception as e:  # transient NRT/axon device errors
            last_exc = e
            import time as _time
            _time.sleep(1.0)
    if res is None:
        raise last_exc
    total = np.float32(0.0)
    for r in res.results:
        total = np.float32(total + np.float32(r["out"][0]) + np.float32(r["out"][1]))
    return np.array([total], dtype=np.float32)


if __name__ == "__main__":
    rng = np.random.default_rng(0)
    p = rng.standard_normal(B).astype(np.float32)
    g = rng.standard_normal(B).astype(np.float32)
    print(kernel(p, g))



# revision 3
# speedup vs baseline: 2.8086x; 2.8086x over previous
"""Trainium2 Bass kernel for DocumentBertScoringLoss (B=8192).

loss = MSE(p, g) + MR(p, g) + SIM(p, g), returned as shape-(1,) fp32.

Margin-ranking identity (ties included):

    sum_{m,n} max(0, 0.1 - r*dp) = 0.1*B^2 - 2*S,
    S = sum_{i<j} min(|p_i - p_j|, 0.1)

S is evaluated on the SORTED prediction vector ps with a banded sweep:
pair (i, j), i<j, is assigned to the 128-row chunk containing i and is
only computed explicitly when j < chunk_start + W.  The host verifies
the band bound  ps[s+W] - ps[s+127] >= 0.1  for every chunk start s
(so every skipped pair has difference >= 0.1 and contributes exactly
0.1, a closed-form count the host adds); if the bound fails, W is
widened (ladder up to the full B, which is the exact dense sweep).

Per chunk the device computes h = min(max(X, s1), s1 + 0.1)
= s1 + clamp(x - s1, 0, 0.1) over the [128, W] window (DVE 4x-mode
tensor_scalar; X is the sorted vector broadcast along partitions in
fp16).  In-window pairs with j <= i clamp to 0, so the full rectangle
sums exactly the i<j near pairs; the linear s1 term is removed with a
512... W*sum(s1)?? no: ncols*s1 with ncols == W per row, subtracted via
one matmul.  Rows past B are padded with -1000 so their clamp is 0 and
the pad columns contribute pure s1 (absorbed by the same correction).

h tiles are reduced column-wise by the PE using h as matmul *weights*
against a ones vector ([128, 128] pieces -> PSUM [128, 1] accumulate),
then one more matmul folds partitions.  MSE/SIM terms: pp/gg via ACT
Square-with-accumulate, dot via one DVE scalar_tensor_tensor; one short
scalar chain assembles the output.  Sharding: core k owns sorted rows
[1024k, 1024k+1024); host gather sums the 8 per-core scalars and adds
the closed-form far-pair constant.
"""

import numpy as np

import concourse.bass as bass
import concourse.bacc as bacc
import concourse.mybir as mybir
from concourse.bass_utils import run_bass_kernel_spmd
from concourse.tile import TileContext
from concourse.alu_op_type import AluOpType

B = 8192
NCORES = 8
ROWS_PER_CORE = B // NCORES          # 1024
NCHUNK = ROWS_PER_CORE // 128        # 8 chunks of 128 partitions
MR_BIAS = 0.1
PAD_VAL = -1000.0

# Band-width ladder: W=640 holds for N(0,1) data with ~10 sigma to
# spare; the tail entries are correctness fallbacks (W=B is the exact
# dense sweep for arbitrary inputs).
W_LADDER = (640, 1024, 1536, 2560, 4096, 8192)

F32 = mybir.dt.float32
F16 = mybir.dt.float16

_CACHED = {}


def _build_nc(W):
    WIN = 128 * (NCHUNK - 1) + W     # per-core broadcast window size
    nc = bacc.Bacc("TRN2", target_bir_lowering=False, debug=False,
                   num_devices=NCORES)

    xwin_d = nc.dram_tensor("x_win", [WIN], F16, kind="ExternalInput")
    prow_d = nc.dram_tensor("p_rows", [128, NCHUNK], F32, kind="ExternalInput")
    p_d = nc.dram_tensor("predictions", [B], F32, kind="ExternalInput")
    g_d = nc.dram_tensor("correct_output", [B], F32, kind="ExternalInput")
    out_d = nc.dram_tensor("out", [2], F32, kind="ExternalOutput")

    AF = mybir.ActivationFunctionType
    K2 = -2.0 / (float(B) * float(B))

    # X broadcast piece split: piece 0 covers chunks 0-3, piece 1 the rest.
    XCUT = 128 * 3 + W

    with TileContext(nc) as tc:
        with (
            tc.tile_pool(name="const", bufs=1) as cpool,
            tc.tile_pool(name="hbuf", bufs=3) as hpool,
            tc.tile_pool(name="psum", bufs=1, space="PSUM") as ppool,
        ):
            xbf = cpool.tile([128, WIN], F16, name="xbf")
            prow = cpool.tile([128, NCHUNK], F32, name="prow")
            s2 = cpool.tile([128, NCHUNK], F32, name="s2")
            p32 = cpool.tile([128, B // 128], F32, name="p32")
            g32 = cpool.tile([128, B // 128], F32, name="g32")
            ones16 = cpool.tile([128, 1], F16, name="ones16")
            ones32 = cpool.tile([128, 1], F32, name="ones32")
            zeros1 = cpool.tile([128, 1], F32, name="zeros1")
            stacked = cpool.tile([128, 3], F32, name="stacked")
            junk_sq = cpool.tile([128, B // 128], F32, name="junk_sq")
            junk_stt = cpool.tile([128, B // 128], F32, name="junk_stt")
            s1sum = cpool.tile([128, 1], F32, name="s1sum")
            v_sb = cpool.tile([128, 1], F32, name="v_sb")
            sc = cpool.tile([1, 16], F32, name="sc")
            out_sb = cpool.tile([1, 2], F32, name="out_sb")

            psum_acc = ppool.tile([128, 1], F32, name="psum_acc")
            psum_small = ppool.tile([1, 3], F32, name="psum_small")
            psum_tot = ppool.tile([1, 1], F32, name="psum_tot")

            # ---- input DMAs (HWDGE is a flat ~630ns serial resource:
            # order by first use) ----
            xw = xwin_d[:]
            nc.sync.dma_start(xbf[:, 0:XCUT], xw[0:XCUT].partition_broadcast(128))
            nc.sync.dma_start(prow, prow_d[:, :])
            nc.scalar.dma_start(p32, p_d[:].rearrange("(p c) -> p c", p=128))
            nc.scalar.dma_start(g32, g_d[:].rearrange("(p c) -> p c", p=128))
            nc.gpsimd.dma_start(
                xbf[:, XCUT:WIN], xw[XCUT:WIN].partition_broadcast(128)
            )

            nc.vector.memset(ones16, 1.0)
            nc.vector.memset(ones32, 1.0)
            nc.vector.memset(zeros1, 0.0)
            # Dummy sqrt first so the single act-table load covers
            # sqrt+square (issued during the DMA wait).
            warm_sqrt = cpool.tile([1, 1], F32, name="warm_sqrt")
            nc.scalar.activation(warm_sqrt, zeros1[0:1, :], AF.Sqrt,
                                 bias=zeros1[0:1, :])

            nc.vector.tensor_scalar(s2, prow, MR_BIAS, None, AluOpType.add)
            nc.vector.tensor_reduce(
                s1sum, prow, mybir.AxisListType.X, AluOpType.add
            )

            # ---- banded clamp sweep: h = min(max(X, s1), s1 + 0.1) ----
            mm = 0
            total_mm = NCHUNK * (W // 128)
            for c in range(NCHUNK):
                h = hpool.tile([128, W], F16, tag="h", name="h", bufs=3)
                nc.vector.tensor_scalar(
                    h, xbf[:, 128 * c:128 * c + W],
                    prow[:, c:c + 1], s2[:, c:c + 1],
                    AluOpType.max, AluOpType.min,
                )
                for j in range(0, W, 128):
                    nc.tensor.matmul(
                        psum_acc, h[:, j:j + 128], ones16,
                        start=(mm == 0), stop=(mm == total_mm - 1),
                        skip_group_check=True,
                    )
                    mm += 1

            # ---- small terms: pp/gg on ACT, dot on DVE ----
            nc.scalar.activation(
                junk_sq, p32, AF.Square, bias=zeros1,
                accum_out=stacked[:, 1:2],
            )
            nc.scalar.activation(
                junk_sq, g32, AF.Square, bias=zeros1,
                accum_out=stacked[:, 2:3],
            )
            nc.vector.scalar_tensor_tensor(
                junk_stt, p32, 1.0, g32, AluOpType.mult, AluOpType.mult,
                accum_out=stacked[:, 0:1],
            )
            nc.tensor.matmul(psum_small, ones32, stacked, start=True, stop=True)

            # ---- scalar chain (partition 0) ----
            smalls = cpool.tile([1, 3], F32, name="smalls")
            nc.vector.tensor_copy(smalls, psum_small)
            dot = smalls[0:1, 0:1]
            pp = smalls[0:1, 1:2]
            gg = smalls[0:1, 2:3]

            prod = sc[0:1, 0:1]
            nc.vector.scalar_tensor_tensor(
                prod, pp, 1.0, gg, AluOpType.mult, AluOpType.mult
            )
            denom = sc[0:1, 1:2]
            nc.scalar.activation(denom, prod, AF.Sqrt, bias=zeros1[0:1, :])
            dmax = sc[0:1, 2:3]
            nc.vector.tensor_scalar(dmax, denom, 1e-8, None, AluOpType.max)
            inv = sc[0:1, 3:4]
            nc.vector.reciprocal(inv, dmax)
            sims = sc[0:1, 4:5]
            nc.vector.scalar_tensor_tensor(
                sims, dot, 1.0, inv, AluOpType.mult, AluOpType.mult
            )
            m1 = sc[0:1, 5:6]
            nc.vector.scalar_tensor_tensor(
                m1, pp, 1.0, gg, AluOpType.mult, AluOpType.add
            )
            mse_raw = sc[0:1, 6:7]
            nc.vector.scalar_tensor_tensor(
                mse_raw, dot, -2.0, m1, AluOpType.mult, AluOpType.add
            )
            # out0 = mse_raw/(8B) + (1 - sims)/8   (host adds the MR
            # far-pair constant)
            e1 = sc[0:1, 7:8]
            nc.vector.tensor_scalar(
                e1, mse_raw, 1.0 / (8.0 * B), 1.0 / 8.0,
                AluOpType.mult, AluOpType.add,
            )
            nc.vector.scalar_tensor_tensor(
                out_sb[0:1, 0:1], sims, -1.0 / 8.0, e1,
                AluOpType.mult, AluOpType.add,
            )

            # ---- MR tail: v = psum_acc - W*s1sum, out1 = K2 * sum(v) ----
            nc.vector.scalar_tensor_tensor(
                v_sb, s1sum, -float(W), psum_acc[:, 0:1],
                AluOpType.mult, AluOpType.add,
            )
            nc.tensor.matmul(psum_tot, v_sb, ones32, start=True, stop=True)
            nc.vector.tensor_scalar(
                out_sb[0:1, 1:2], psum_tot[0:1, 0:1], K2, None, AluOpType.mult
            )
            nc.sync.dma_start(out_d[None, :], out_sb)

    nc.compile()
    return nc


def _pick_w(ps):
    starts = np.arange(0, B, 128)
    for W in W_LADDER:
        s = starts[starts + W < B]
        if s.size == 0 or np.all(ps[s + W] - ps[s + 127] >= MR_BIAS):
            return W
    return B


def kernel(predictions: np.ndarray, correct_output: np.ndarray) -> np.ndarray:
    p = np.ascontiguousarray(np.asarray(predictions, dtype=np.float32))
    g = np.ascontiguousarray(np.asarray(correct_output, dtype=np.float32))

    ps = np.sort(p)
    W = _pick_w(ps)
    if W not in _CACHED:
        _CACHED[W] = _build_nc(W)
    nc = _CACHED[W]

    WIN = 128 * (NCHUNK - 1) + W
    ps16 = np.full(B + WIN, PAD_VAL, dtype=np.float16)
    ps16[:B] = ps.astype(np.float16)

    in_maps = []
    for k in range(NCORES):
        r0 = k * ROWS_PER_CORE
        in_maps.append(
            {
                "x_win": ps16[r0:r0 + WIN].copy(),
                "p_rows": np.ascontiguousarray(
                    ps[r0:r0 + ROWS_PER_CORE].reshape(NCHUNK, 128).T
                ),
                "predictions": p,
                "correct_output": g,
            }
        )

    res = None
    last_exc = None
    for _attempt in range(3):
        try:
            res = run_bass_kernel_spmd(nc, in_maps, core_ids=list(range(NCORES)))
            break
        except Exception as e:  # transient NRT/axon device errors
            last_exc = e
            import time as _time
            _time.sleep(1.0)
    if res is None:
        raise last_exc

    # Host gather: sum per-core scalars + closed-form far-pair constant.
    # N_near counts pairs (i, j), i<j, j < 128*(i//128) + W (clipped at B);
    # skipped pairs contribute exactly 0.1 each:
    #   mr = 0.1 - (2/B^2) * (S_near + 0.1*N_far)
    # device out[1] already carries -(2/B^2)*S_near_share; out[0] carries
    # (mse + sim)/8; the remaining constant is added here.
    i = np.arange(B, dtype=np.int64)
    hi = np.minimum(128 * (i // 128) + W, B)
    n_near = int(np.sum(hi - i - 1))
    n_far = B * (B - 1) // 2 - n_near
    mr_const = MR_BIAS - 2.0 * MR_BIAS * n_far / (float(B) * float(B))

    total = np.float64(mr_const)
    for r in res.results:
        total += np.float64(r["out"][0]) + np.float64(r["out"][1])
    return np.array([total], dtype=np.float32)


if __name__ == "__main__":
    rng = np.random.default_rng(0)
    p = rng.standard_normal(B).astype(np.float32)
    g = rng.standard_normal(B).astype(np.float32)
    print(kernel(p, g))


# revision 7
# speedup vs baseline: 3.0338x; 1.0802x over previous
"""Trainium2 Bass kernel for DocumentBertScoringLoss (B=8192).

loss = MSE(p, g) + MR(p, g) + SIM(p, g), returned as shape-(1,) fp32.

Margin-ranking identity (ties included):

    sum_{m,n} max(0, 0.1 - r*dp) = 0.1*B^2 - 2*S,
    S = sum_{i<j} min(p_(j) - p_(i), 0.1)   on the sorted predictions.

S is evaluated with a banded sweep over the sorted vector: pair (i, j),
i<j, is assigned to the 128-row chunk containing i and computed
explicitly only when j < chunk_start + W.  The host verifies the band
bound  ps[s+W] - ps[s+127] >= 0.1  for every chunk start s (every
skipped pair then differs by >= 0.1 and contributes exactly 0.1, a
closed-form count added on the host); if the bound fails, W widens
(ladder up to W=B, the exact dense sweep, so the kernel is correct for
arbitrary inputs).  In-window pairs with j <= i clamp to 0 and rows
past B are padded with -1000 (clamp 0), so each 128xW rectangle sums
exactly its i<j near pairs plus W*s1 per row, removed via one
correction.

Device pipeline per core (1024 sorted rows):
  - DVE: 8x tensor_scalar h = min(max(X, s1), s1+0.1) over [128, W]
    fp16 windows (4x perf mode, ~0.26 ns/col).
  - PE reduces each h column-wise using h as matmul *weights* against a
    ones vector ([128,128] pieces accumulating into PSUM [128,1]), then
    folds partitions with two more 1-col matmuls.
  - MSE/SIM: dot/pp/gg via 3 scalar_tensor_tensor accumulates off the
    packed p||g tile; 1/sqrt(pp*gg) via one ACT Rsqrt (single act-table
    load); short DVE scalar chain reading PSUM directly.
  - DMAs: X broadcast in 3 pieces + prow/pg/out spread over the two
    HWDGE queues and Pool SWDGE to overlap the ~2.2us per-DMA latency
    (desc-gen + dge delay + 900ns completion semaphore).
Host gather sums the 8 per-core scalars and adds the far-pair constant.
"""

import numpy as np

import concourse.bass as bass
import concourse.bacc as bacc
import concourse.mybir as mybir
from concourse.bass_utils import run_bass_kernel_spmd
from concourse.tile import TileContext
from concourse.alu_op_type import AluOpType

B = 8192
NCORES = 8
ROWS_PER_CORE = B // NCORES          # 1024
NCHUNK = ROWS_PER_CORE // 128        # 8 chunks of 128 partitions
MR_BIAS = 0.1
PAD_VAL = -1000.0

# Band-width ladder: W=512 holds for the reference N(0,1) draw (host
# verified per call); later entries are correctness fallbacks (W=B is
# the exact dense sweep).
W_LADDER = (512, 640, 1024, 1536, 2560, 4096, 8192)

F32 = mybir.dt.float32
F16 = mybir.dt.float16

_CACHED = {}


def _build_nc(W):
    WIN = 128 * (NCHUNK - 1) + W     # per-core broadcast window size
    nc = bacc.Bacc("TRN2", target_bir_lowering=False, debug=False,
                   num_devices=NCORES)

    xwin_d = nc.dram_tensor("x_win", [WIN], F16, kind="ExternalInput")
    prow_d = nc.dram_tensor("p_rows", [128, NCHUNK], F32, kind="ExternalInput")
    pg_d = nc.dram_tensor("pg", [128, 128], F32, kind="ExternalInput")
    out_d = nc.dram_tensor("out", [2], F32, kind="ExternalOutput")

    AF = mybir.ActivationFunctionType
    K2 = -2.0 / (float(B) * float(B))

    # X piece split: A covers chunk 0's window, B chunks 1-3, C the rest.
    CUT1 = W + 128
    CUT2 = W + 128 * 4

    with TileContext(nc) as tc:
        with (
            tc.tile_pool(name="const", bufs=1) as cpool,
            tc.tile_pool(name="hbuf", bufs=3) as hpool,
            tc.tile_pool(name="psum", bufs=1, space="PSUM") as ppool,
        ):
            xbf = cpool.tile([128, WIN], F16, name="xbf")
            prow = cpool.tile([128, NCHUNK], F32, name="prow")
            s2 = cpool.tile([128, NCHUNK], F32, name="s2")
            pg32 = cpool.tile([128, 128], F32, name="pg32")
            ones16 = cpool.tile([128, 1], F16, name="ones16")
            ones32 = cpool.tile([128, 1], F32, name="ones32")
            zeros1 = cpool.tile([128, 1], F32, name="zeros1")
            stacked = cpool.tile([128, 3], F32, name="stacked")
            junk_stt = cpool.tile([128, 64], F32, name="junk_stt")
            s1sum = cpool.tile([128, 1], F32, name="s1sum")
            v_sb = cpool.tile([128, 1], F32, name="v_sb")
            sc = cpool.tile([1, 16], F32, name="sc")
            out_sb = cpool.tile([1, 2], F32, name="out_sb")

            psum_acc = ppool.tile([128, 1], F32, name="psum_acc")
            psum_small = ppool.tile([1, 3], F32, name="psum_small")
            psum_tot = ppool.tile([1, 1], F32, name="psum_tot")

            # ---- input DMAs ----
            # HWDGE (one serial ~630ns/desc-gen resource fed by the SP and
            # ACT queues): X pieces in consumption order.  Pool SWDGE
            # (idle engine) carries prow, pg, and preps the out DMA early.
            xw = xwin_d[:]
            nc.sync.dma_start(xbf[:, 0:W], xw[0:W].partition_broadcast(128))
            nc.scalar.dma_start(
                xbf[:, W:CUT2], xw[W:CUT2].partition_broadcast(128)
            )
            nc.sync.dma_start(
                xbf[:, CUT2:WIN], xw[CUT2:WIN].partition_broadcast(128)
            )
            nc.gpsimd.dma_start(prow, prow_d[:, :])
            nc.gpsimd.dma_start(pg32, pg_d[:, :])

            nc.vector.memset(ones16, 1.0)
            nc.vector.memset(ones32, 1.0)
            nc.vector.memset(zeros1, 0.0)
            k2vec = cpool.tile([128, 1], F32, name="k2vec")
            nc.vector.memset(k2vec, K2)
            # Dummy sqrt so the single act-table load (during the DMA
            # wait) covers the one real ACT op.
            warm_rs = cpool.tile([1, 1], F32, name="warm_rs")
            nc.scalar.activation(warm_rs, ones32[0:1, :], AF.Sqrt,
                                 bias=zeros1[0:1, :])

            # ---- banded clamp sweep (kept ahead of everything else in
            # the DVE stream via scheduler priority) ----
            mm = 0
            total_mm = NCHUNK * (W // 128)
            with tc.high_priority():
                nc.vector.tensor_scalar(s2, prow, MR_BIAS, None, AluOpType.add)
                nc.vector.tensor_reduce(
                    s1sum, prow, mybir.AxisListType.X, AluOpType.add
                )
                for c in range(NCHUNK):
                    h = hpool.tile([128, W], F16, tag="h", name="h", bufs=3)
                    nc.vector.tensor_scalar(
                        h, xbf[:, 128 * c:128 * c + W],
                        prow[:, c:c + 1], s2[:, c:c + 1],
                        AluOpType.max, AluOpType.min,
                    )
                    for j in range(0, W, 128):
                        nc.tensor.matmul(
                            psum_acc, h[:, j:j + 128], ones16,
                            start=(mm == 0), stop=(mm == total_mm - 1),
                            skip_group_check=True,
                        )
                        mm += 1

            # ---- MR tail: v = psum_acc - W*s1sum, out1 = K2 * sum(v)
            # (K2 folded into the reduction vector; the final PSUM read is
            # a plain copy since stt/ts cannot read PSUM here) ----
            acc_sb = cpool.tile([128, 1], F32, name="acc_sb")
            nc.vector.tensor_copy(acc_sb, psum_acc)
            nc.vector.scalar_tensor_tensor(
                v_sb, s1sum, -float(W), acc_sb,
                AluOpType.mult, AluOpType.add,
            )
            nc.tensor.matmul(psum_tot, v_sb, k2vec, start=True, stop=True)
            nc.vector.tensor_copy(out_sb[0:1, 1:2], psum_tot[0:1, 0:1])

            # ---- small terms: dot/pp/gg off the packed p||g tile ----
            p_ap = pg32[:, 0:64]
            g_ap = pg32[:, 64:128]
            nc.vector.scalar_tensor_tensor(
                junk_stt, p_ap, 1.0, g_ap, AluOpType.mult, AluOpType.mult,
                accum_out=stacked[:, 0:1],
            )
            nc.vector.scalar_tensor_tensor(
                junk_stt, p_ap, 1.0, p_ap, AluOpType.mult, AluOpType.mult,
                accum_out=stacked[:, 1:2],
            )
            nc.vector.scalar_tensor_tensor(
                junk_stt, g_ap, 1.0, g_ap, AluOpType.mult, AluOpType.mult,
                accum_out=stacked[:, 2:3],
            )
            nc.tensor.matmul(psum_small, ones32, stacked, start=True, stop=True)

            # ---- scalar chain (partition 0) ----
            smalls = cpool.tile([1, 3], F32, name="smalls")
            nc.vector.tensor_copy(smalls, psum_small)
            dot = smalls[0:1, 0:1]
            pp = smalls[0:1, 1:2]
            gg = smalls[0:1, 2:3]
            prod = sc[0:1, 0:1]
            nc.vector.scalar_tensor_tensor(
                prod, pp, 1.0, gg, AluOpType.mult, AluOpType.mult
            )
            prodc = sc[0:1, 1:2]
            nc.vector.tensor_scalar(prodc, prod, 1e-16, None, AluOpType.max)
            denom = sc[0:1, 7:8]
            nc.scalar.activation(denom, prodc, AF.Sqrt, bias=zeros1[0:1, :])
            inv = sc[0:1, 2:3]
            nc.vector.reciprocal(inv, denom)
            m1 = sc[0:1, 3:4]
            nc.vector.scalar_tensor_tensor(
                m1, pp, 1.0, gg, AluOpType.mult, AluOpType.add
            )
            mse_raw = sc[0:1, 4:5]
            nc.vector.scalar_tensor_tensor(
                mse_raw, dot, -2.0, m1, AluOpType.mult, AluOpType.add
            )
            e1 = sc[0:1, 5:6]
            nc.vector.tensor_scalar(
                e1, mse_raw, 1.0 / (8.0 * B), 1.0 / 8.0,
                AluOpType.mult, AluOpType.add,
            )
            sims = sc[0:1, 6:7]
            nc.vector.scalar_tensor_tensor(
                sims, dot, 1.0, inv, AluOpType.mult, AluOpType.mult
            )
            nc.vector.scalar_tensor_tensor(
                out_sb[0:1, 0:1], sims, -1.0 / 8.0, e1,
                AluOpType.mult, AluOpType.add,
            )
            nc.gpsimd.dma_start(out_d[None, :], out_sb)

    nc.compile()
    return nc


def _pick_w(ps):
    starts = np.arange(0, B, 128)
    for W in W_LADDER:
        s = starts[starts + W < B]
        if s.size == 0 or np.all(ps[s + W] - ps[s + 127] >= MR_BIAS):
            return W
    return B


def kernel(predictions: np.ndarray, correct_output: np.ndarray) -> np.ndarray:
    p = np.ascontiguousarray(np.asarray(predictions, dtype=np.float32))
    g = np.ascontiguousarray(np.asarray(correct_output, dtype=np.float32))

    ps = np.sort(p)
    W = _pick_w(ps)
    if W not in _CACHED:
        _CACHED[W] = _build_nc(W)
    nc = _CACHED[W]

    WIN = 128 * (NCHUNK - 1) + W
    ps16 = np.full(B + WIN, PAD_VAL, dtype=np.float16)
    ps16[:B] = ps.astype(np.float16)
    pg = np.concatenate(
        [p.reshape(128, 64), g.reshape(128, 64)], axis=1
    ).astype(np.float32)

    in_maps = []
    for k in range(NCORES):
        r0 = k * ROWS_PER_CORE
        in_maps.append(
            {
                "x_win": ps16[r0:r0 + WIN].copy(),
                "p_rows": np.ascontiguousarray(
                    ps[r0:r0 + ROWS_PER_CORE].reshape(NCHUNK, 128).T
                ),
                "pg": pg,
            }
        )

    res = None
    last_exc = None
    for _attempt in range(3):
        try:
            res = run_bass_kernel_spmd(nc, in_maps, core_ids=list(range(NCORES)))
            break
        except Exception as e:  # transient NRT/axon device errors
            last_exc = e
            import time as _time
            _time.sleep(1.0)
    if res is None:
        raise last_exc

    # Host gather: per-core scalars + closed-form far-pair constant.
    #   mr = 0.1 - (2/B^2) * (S_near + 0.1*N_far)
    # device out[1] carries -(2/B^2)*S_near_share, out[0] (mse+sim)/8.
    i = np.arange(B, dtype=np.int64)
    hi = np.minimum(128 * (i // 128) + W, B)
    n_near = int(np.sum(hi - i - 1))
    n_far = B * (B - 1) // 2 - n_near
    mr_const = MR_BIAS - 2.0 * MR_BIAS * n_far / (float(B) * float(B))

    total = np.float64(mr_const)
    for r in res.results:
        total += np.float64(r["out"][0]) + np.float64(r["out"][1])
    return np.array([total], dtype=np.float32)


if __name__ == "__main__":
    rng = np.random.default_rng(0)
    p = rng.standard_normal(B).astype(np.float32)
    g = rng.standard_normal(B).astype(np.float32)
    print(kernel(p, g))


# revision 8
# speedup vs baseline: 3.2932x; 1.0855x over previous
"""Trainium2 Bass kernel for DocumentBertScoringLoss (B=8192).

loss = MSE(p, g) + MR(p, g) + SIM(p, g), returned as shape-(1,) fp32.

Margin-ranking identity (ties included):

    sum_{m,n} max(0, 0.1 - r*dp) = 0.1*B^2 - 2*S,
    S = sum_{i<j} min(p_(j) - p_(i), 0.1)   on the sorted predictions.

S is evaluated with a banded sweep over the sorted vector: pair (i, j),
i<j, is assigned to the 128-row chunk containing i and computed
explicitly only when j < chunk_start + W.  The host verifies the band
bound  ps[s+W] - ps[s+127] >= 0.1  for every chunk start s (every
skipped pair then differs by >= 0.1 and contributes exactly 0.1, a
closed-form count added on the host); if the bound fails, W widens
(ladder up to W=B, the exact dense sweep, so the kernel is correct for
arbitrary inputs).  In-window pairs with j <= i clamp to 0 and rows
past B are padded with -1000 (clamp 0), so each 128xW rectangle sums
exactly its i<j near pairs plus W*s1 per row, removed via one
correction.

Device pipeline per core (1024 sorted rows):
  - DVE: 8x tensor_scalar h = min(max(X, s1), s1+0.1) over [128, W]
    fp16 windows (4x perf mode, ~0.26 ns/col).
  - PE reduces each h column-wise using h as matmul *weights* against a
    ones vector ([128,128] pieces accumulating into PSUM [128,1]), then
    folds partitions with two more 1-col matmuls.
  - MSE/SIM: dot/pp/gg via 3 scalar_tensor_tensor accumulates off the
    packed p||g tile; 1/sqrt(pp*gg) via one ACT Rsqrt (single act-table
    load); short DVE scalar chain reading PSUM directly.
  - DMAs: X broadcast in 3 pieces + prow/pg/out spread over the two
    HWDGE queues and Pool SWDGE to overlap the ~2.2us per-DMA latency
    (desc-gen + dge delay + 900ns completion semaphore).
Host gather sums the 8 per-core scalars and adds the far-pair constant.
"""

import numpy as np

import concourse.bass as bass
import concourse.bacc as bacc
import concourse.mybir as mybir
from concourse.bass_utils import run_bass_kernel_spmd
from concourse.tile import TileContext
from concourse.alu_op_type import AluOpType

B = 8192
NCORES = 8
ROWS_PER_CORE = B // NCORES          # 1024
NCHUNK = ROWS_PER_CORE // 128        # 8 chunks of 128 partitions
MR_BIAS = 0.1
PAD_VAL = -1000.0

# Band-width ladder: W=512 holds for the reference N(0,1) draw (host
# verified per call); later entries are correctness fallbacks (W=B is
# the exact dense sweep).
W_LADDER = (512, 640, 1024, 1536, 2560, 4096, 8192)

F32 = mybir.dt.float32
F16 = mybir.dt.float16

_CACHED = {}


def _build_nc(W):
    WIN = 128 * (NCHUNK - 1) + W     # per-core broadcast window size
    nc = bacc.Bacc("TRN2", target_bir_lowering=False, debug=False,
                   num_devices=NCORES)

    xwin_d = nc.dram_tensor("x_win", [WIN], F16, kind="ExternalInput")
    prow_d = nc.dram_tensor("p_rows", [128, NCHUNK], F32, kind="ExternalInput")
    pg_d = nc.dram_tensor("pg", [128, 256], F16, kind="ExternalInput")
    out_d = nc.dram_tensor("out", [2], F32, kind="ExternalOutput")

    AF = mybir.ActivationFunctionType
    K2 = -2.0 / (float(B) * float(B))

    # X piece split: A covers chunks 0-2, B chunks 3-7 (bulk), C the tail.
    CUT1 = W + 128 * 2
    CUT2 = W + 128 * 6

    with TileContext(nc) as tc:
        with (
            tc.tile_pool(name="const", bufs=1) as cpool,
            tc.tile_pool(name="hbuf", bufs=3) as hpool,
            tc.tile_pool(name="psum", bufs=1, space="PSUM") as ppool,
        ):
            xbf = cpool.tile([128, WIN], F16, name="xbf")
            prow = cpool.tile([128, NCHUNK], F32, name="prow")
            s2 = cpool.tile([128, NCHUNK], F32, name="s2")
            pg16 = cpool.tile([128, 256], F16, name="pg16")
            ones16 = cpool.tile([128, 1], F16, name="ones16")
            ones32 = cpool.tile([128, 1], F32, name="ones32")
            zeros1 = cpool.tile([128, 1], F32, name="zeros1")
            stacked = cpool.tile([128, 3], F32, name="stacked")
            junk_stt = cpool.tile([128, 64], F16, name="junk_stt")
            s1sum = cpool.tile([128, 1], F32, name="s1sum")
            v_sb = cpool.tile([128, 1], F32, name="v_sb")
            sc = cpool.tile([1, 16], F32, name="sc")
            out_sb = cpool.tile([1, 2], F32, name="out_sb")

            psum_acc = ppool.tile([128, 1], F32, name="psum_acc")
            psum_small = ppool.tile([1, 3], F32, name="psum_small")
            psum_tot = ppool.tile([1, 1], F32, name="psum_tot")

            # ---- input DMAs ----
            # HWDGE (one serial ~630ns/desc-gen resource fed by the SP and
            # ACT queues): X pieces in consumption order.  Pool SWDGE
            # (idle engine) carries prow, pg, and preps the out DMA early.
            xw = xwin_d[:]
            nc.sync.dma_start(xbf[:, 0:CUT1], xw[0:CUT1].partition_broadcast(128))
            nc.scalar.dma_start(
                xbf[:, CUT1:CUT2], xw[CUT1:CUT2].partition_broadcast(128)
            )
            nc.sync.dma_start(
                xbf[:, CUT2:WIN], xw[CUT2:WIN].partition_broadcast(128)
            )
            nc.gpsimd.dma_start(prow, prow_d[:, :])
            nc.gpsimd.dma_start(pg16, pg_d[:, :])

            nc.vector.memset(ones16, 1.0)
            nc.vector.memset(ones32, 1.0)
            nc.vector.memset(zeros1, 0.0)
            k2vec = cpool.tile([128, 1], F32, name="k2vec")
            nc.vector.memset(k2vec, K2)
            # Dummy sqrt so the single act-table load (during the DMA
            # wait) covers the one real ACT op.
            warm_rs = cpool.tile([1, 1], F32, name="warm_rs")
            nc.scalar.activation(warm_rs, ones32[0:1, :], AF.Sqrt,
                                 bias=zeros1[0:1, :])

            # ---- banded clamp sweep (kept ahead of everything else in
            # the DVE stream via scheduler priority) ----
            mm = 0
            total_mm = NCHUNK * (W // 128)
            with tc.high_priority():
                nc.vector.tensor_scalar(s2, prow, MR_BIAS, None, AluOpType.add)
                for c in range(NCHUNK):
                    h = hpool.tile([128, W], F16, tag="h", name="h", bufs=8)
                    nc.vector.tensor_scalar(
                        h, xbf[:, 128 * c:128 * c + W],
                        prow[:, c:c + 1], s2[:, c:c + 1],
                        AluOpType.max, AluOpType.min,
                    )
                    for j in range(0, W, 128):
                        nc.tensor.matmul(
                            psum_acc, h[:, j:j + 128], ones16,
                            start=(mm == 0), stop=(mm == total_mm - 1),
                            skip_group_check=True,
                        )
                        mm += 1

            # ---- MR tail: v = psum_acc - W*s1sum, out1 = K2 * sum(v)
            # (K2 folded into the reduction vector; the final PSUM read is
            # a plain copy since stt/ts cannot read PSUM here) ----
            nc.vector.tensor_reduce(
                s1sum, prow, mybir.AxisListType.X, AluOpType.add
            )
            acc_sb = cpool.tile([128, 1], F32, name="acc_sb")
            nc.vector.tensor_copy(acc_sb, psum_acc)
            nc.vector.scalar_tensor_tensor(
                v_sb, s1sum, -float(W), acc_sb,
                AluOpType.mult, AluOpType.add,
            )
            nc.tensor.matmul(psum_tot, v_sb, k2vec, start=True, stop=True)
            nc.vector.tensor_copy(out_sb[0:1, 1:2], psum_tot[0:1, 0:1])

            # ---- small terms: dot/pp/gg off the packed p||g tile ----
            p_ap = pg16[:, 0:64]
            g_ap = pg16[:, 64:128]
            nc.vector.scalar_tensor_tensor(
                junk_stt, p_ap, 1.0, g_ap, AluOpType.mult, AluOpType.mult,
                accum_out=stacked[:, 0:1],
            )
            nc.vector.scalar_tensor_tensor(
                junk_stt, p_ap, 1.0, p_ap, AluOpType.mult, AluOpType.mult,
                accum_out=stacked[:, 1:2],
            )
            nc.vector.scalar_tensor_tensor(
                junk_stt, g_ap, 1.0, g_ap, AluOpType.mult, AluOpType.mult,
                accum_out=stacked[:, 2:3],
            )
            nc.tensor.matmul(psum_small, ones32, stacked, start=True, stop=True)

            # ---- scalar chain (partition 0) ----
            smalls = cpool.tile([1, 3], F32, name="smalls")
            nc.vector.tensor_copy(smalls, psum_small)
            dot = smalls[0:1, 0:1]
            pp = smalls[0:1, 1:2]
            gg = smalls[0:1, 2:3]
            prod = sc[0:1, 0:1]
            nc.vector.scalar_tensor_tensor(
                prod, pp, 1.0, gg, AluOpType.mult, AluOpType.mult
            )
            prodc = sc[0:1, 1:2]
            nc.vector.tensor_scalar(prodc, prod, 1e-16, None, AluOpType.max)
            denom = sc[0:1, 7:8]
            nc.scalar.activation(denom, prodc, AF.Sqrt, bias=zeros1[0:1, :])
            inv = sc[0:1, 2:3]
            nc.vector.reciprocal(inv, denom)
            m1 = sc[0:1, 3:4]
            nc.vector.scalar_tensor_tensor(
                m1, pp, 1.0, gg, AluOpType.mult, AluOpType.add
            )
            mse_raw = sc[0:1, 4:5]
            nc.vector.scalar_tensor_tensor(
                mse_raw, dot, -2.0, m1, AluOpType.mult, AluOpType.add
            )
            e1 = sc[0:1, 5:6]
            nc.vector.tensor_scalar(
                e1, mse_raw, 1.0 / (8.0 * B), 1.0 / 8.0,
                AluOpType.mult, AluOpType.add,
            )
            sims = sc[0:1, 6:7]
            nc.vector.scalar_tensor_tensor(
                sims, dot, 1.0, inv, AluOpType.mult, AluOpType.mult
            )
            nc.vector.scalar_tensor_tensor(
                out_sb[0:1, 0:1], sims, -1.0 / 8.0, e1,
                AluOpType.mult, AluOpType.add,
            )
            nc.sync.dma_start(out_d[None, :], out_sb)

    nc.compile()
    return nc


def _pick_w(ps):
    starts = np.arange(0, B, 128)
    for W in W_LADDER:
        s = starts[starts + W < B]
        if s.size == 0 or np.all(ps[s + W] - ps[s + 127] >= MR_BIAS):
            return W
    return B


def kernel(predictions: np.ndarray, correct_output: np.ndarray) -> np.ndarray:
    p = np.ascontiguousarray(np.asarray(predictions, dtype=np.float32))
    g = np.ascontiguousarray(np.asarray(correct_output, dtype=np.float32))

    ps = np.sort(p)
    W = _pick_w(ps)
    if W not in _CACHED:
        _CACHED[W] = _build_nc(W)
    nc = _CACHED[W]

    WIN = 128 * (NCHUNK - 1) + W
    ps16 = np.full(B + WIN, PAD_VAL, dtype=np.float16)
    ps16[:B] = ps.astype(np.float16)
    pg = np.zeros((128, 256), dtype=np.float16)
    pg[:, 0:64] = p.reshape(128, 64)
    pg[:, 64:128] = g.reshape(128, 64)

    in_maps = []
    for k in range(NCORES):
        r0 = k * ROWS_PER_CORE
        in_maps.append(
            {
                "x_win": ps16[r0:r0 + WIN].copy(),
                "p_rows": np.ascontiguousarray(
                    ps[r0:r0 + ROWS_PER_CORE].reshape(NCHUNK, 128).T
                ),
                "pg": pg,
            }
        )

    res = None
    last_exc = None
    for _attempt in range(3):
        try:
            res = run_bass_kernel_spmd(nc, in_maps, core_ids=list(range(NCORES)))
            break
        except Exception as e:  # transient NRT/axon device errors
            last_exc = e
            import time as _time
            _time.sleep(1.0)
    if res is None:
        raise last_exc

    # Host gather: per-core scalars + closed-form far-pair constant.
    #   mr = 0.1 - (2/B^2) * (S_near + 0.1*N_far)
    # device out[1] carries -(2/B^2)*S_near_share, out[0] (mse+sim)/8.
    i = np.arange(B, dtype=np.int64)
    hi = np.minimum(128 * (i // 128) + W, B)
    n_near = int(np.sum(hi - i - 1))
    n_far = B * (B - 1) // 2 - n_near
    mr_const = MR_BIAS - 2.0 * MR_BIAS * n_far / (float(B) * float(B))

    total = np.float64(mr_const)
    for r in res.results:
        total += np.float64(r["out"][0]) + np.float64(r["out"][1])
    return np.array([total], dtype=np.float32)


if __name__ == "__main__":
    rng = np.random.default_rng(0)
    p = rng.standard_normal(B).astype(np.float32)
    g = rng.standard_normal(B).astype(np.float32)
    print(kernel(p, g))


# revision 9
# speedup vs baseline: 3.4865x; 1.0587x over previous
"""Trainium2 Bass kernel for DocumentBertScoringLoss (B=8192).

loss = MSE(p, g) + MR(p, g) + SIM(p, g), returned as shape-(1,) fp32.

Margin-ranking identity (ties included):

    sum_{m,n} max(0, 0.1 - r*dp) = 0.1*B^2 - 2*S,
    S = sum_{i<j} min(p_(j) - p_(i), 0.1)   on the sorted predictions.

S is evaluated with a banded sweep over the sorted vector: pair (i, j),
i<j, is assigned to the 128-row chunk containing i and computed
explicitly only when j < chunk_start + W.  The host verifies the band
bound  ps[s+W] - ps[s+127] >= 0.1  for every chunk start s (every
skipped pair then differs by >= 0.1 and contributes exactly 0.1, a
closed-form count added on the host); if the bound fails, W widens
(ladder up to W=B, the exact dense sweep, so the kernel is correct for
arbitrary inputs).  In-window pairs with j <= i clamp to 0 and rows
past B are padded with -1000 (clamp 0), so each 128xW rectangle sums
exactly its i<j near pairs plus W*s1 per row, removed via one
correction.

Device pipeline per core (1024 sorted rows):
  - DVE: 8x tensor_scalar h = min(max(X, s1), s1+0.1) over [128, W]
    fp16 windows (4x perf mode, ~0.26 ns/col).
  - PE reduces each h column-wise using h as matmul *weights* against a
    ones vector ([128,128] pieces accumulating into PSUM [128,1]), then
    folds partitions with two more 1-col matmuls.
  - MSE/SIM: dot/pp/gg via 3 scalar_tensor_tensor accumulates off the
    packed p||g tile; 1/sqrt(pp*gg) via one ACT Rsqrt (single act-table
    load); short DVE scalar chain reading PSUM directly.
  - DMAs: X broadcast in 3 pieces + prow/pg/out spread over the two
    HWDGE queues and Pool SWDGE to overlap the ~2.2us per-DMA latency
    (desc-gen + dge delay + 900ns completion semaphore).
Host gather sums the 8 per-core scalars and adds the far-pair constant.
"""

import numpy as np

import concourse.bass as bass
import concourse.bacc as bacc
import concourse.mybir as mybir
from concourse.bass_utils import run_bass_kernel_spmd
from concourse.tile import TileContext
from concourse.alu_op_type import AluOpType

B = 8192
NCORES = 8
ROWS_PER_CORE = B // NCORES          # 1024
NCHUNK = ROWS_PER_CORE // 128        # 8 chunks of 128 partitions
MR_BIAS = 0.1
PAD_VAL = -1000.0

# Band-width ladder: W=512 holds for the reference N(0,1) draw (host
# verified per call); later entries are correctness fallbacks (W=B is
# the exact dense sweep).
W_LADDER = (512, 640, 1024, 1536, 2560, 4096, 8192)

F32 = mybir.dt.float32
F16 = mybir.dt.float16

_CACHED = {}


def _build_nc(W):
    WIN = 128 * (NCHUNK - 1) + W     # per-core broadcast window size
    nc = bacc.Bacc("TRN2", target_bir_lowering=False, debug=False,
                   num_devices=NCORES)

    xwin_d = nc.dram_tensor("x_win", [WIN], F16, kind="ExternalInput")
    prow_d = nc.dram_tensor("p_rows", [128, 2 * NCHUNK], F32, kind="ExternalInput")
    pg_d = nc.dram_tensor("pg", [128, 256], F16, kind="ExternalInput")
    out_d = nc.dram_tensor("out", [2], F32, kind="ExternalOutput")

    AF = mybir.ActivationFunctionType
    K2 = -2.0 / (float(B) * float(B))

    # X piece split: A covers chunks 0-2, B chunks 3-7 (bulk), C the tail.
    CUT1 = W + 128 * 2
    CUT2 = W + 128 * 6

    with TileContext(nc) as tc:
        with (
            tc.tile_pool(name="const", bufs=1) as cpool,
            tc.tile_pool(name="hbuf", bufs=3) as hpool,
            tc.tile_pool(name="psum", bufs=1, space="PSUM") as ppool,
        ):
            xbf = cpool.tile([128, WIN], F16, name="xbf")
            prow = cpool.tile([128, 2 * NCHUNK], F32, name="prow")
            pg16 = cpool.tile([128, 256], F16, name="pg16")
            ones16 = cpool.tile([128, 1], F16, name="ones16")
            ones32 = cpool.tile([128, 1], F32, name="ones32")
            zeros1 = cpool.tile([128, 1], F32, name="zeros1")
            stacked = cpool.tile([128, 3], F32, name="stacked")
            junk_stt = cpool.tile([128, 64], F16, name="junk_stt")
            s1sum = cpool.tile([128, 1], F32, name="s1sum")
            v_sb = cpool.tile([128, 1], F32, name="v_sb")
            sc = cpool.tile([1, 16], F32, name="sc")
            out_sb = cpool.tile([1, 2], F32, name="out_sb")

            psum_acc = ppool.tile([128, 1], F32, name="psum_acc")
            psum_small = ppool.tile([1, 3], F32, name="psum_small")
            psum_tot = ppool.tile([1, 1], F32, name="psum_tot")

            # ---- input DMAs ----
            # HWDGE (one serial ~630ns/desc-gen resource fed by the SP and
            # ACT queues): X pieces in consumption order.  Pool SWDGE
            # (idle engine) carries prow, pg, and preps the out DMA early.
            xw = xwin_d[:]
            nc.sync.dma_start(xbf[:, 0:CUT1], xw[0:CUT1].partition_broadcast(128))
            nc.scalar.dma_start(
                xbf[:, CUT1:CUT2], xw[CUT1:CUT2].partition_broadcast(128)
            )
            nc.gpsimd.dma_start(prow, prow_d[:, :])
            nc.gpsimd.dma_start(
                xbf[:, CUT2:WIN], xw[CUT2:WIN].partition_broadcast(128)
            )
            nc.sync.dma_start(pg16, pg_d[:, :])

            nc.vector.memset(ones16, 1.0)
            nc.vector.memset(ones32, 1.0)
            nc.vector.memset(zeros1, 0.0)
            k2vec = cpool.tile([128, 1], F32, name="k2vec")
            nc.vector.memset(k2vec, K2)
            # Dummy sqrt so the single act-table load (during the DMA
            # wait) covers the one real ACT op.
            warm_rs = cpool.tile([1, 1], F32, name="warm_rs")
            nc.scalar.activation(warm_rs, ones32[0:1, :], AF.Sqrt,
                                 bias=zeros1[0:1, :])

            # ---- banded clamp sweep (kept ahead of everything else in
            # the DVE stream via scheduler priority) ----
            mm = 0
            total_mm = NCHUNK * (W // 128)
            with tc.high_priority():
                for c in range(NCHUNK):
                    h = hpool.tile([128, W], F16, tag="h", name="h", bufs=8)
                    nc.vector.tensor_scalar(
                        h, xbf[:, 128 * c:128 * c + W],
                        prow[:, c:c + 1], prow[:, NCHUNK + c:NCHUNK + c + 1],
                        AluOpType.max, AluOpType.min,
                    )
                    for j in range(0, W, 128):
                        nc.tensor.matmul(
                            psum_acc, h[:, j:j + 128], ones16,
                            start=(mm == 0), stop=(mm == total_mm - 1),
                            skip_group_check=True,
                        )
                        mm += 1

            # ---- MR tail: v = psum_acc - W*s1sum, out1 = K2 * sum(v)
            # (K2 folded into the reduction vector; the final PSUM read is
            # a plain copy since stt/ts cannot read PSUM here) ----
            nc.vector.tensor_reduce(
                s1sum, prow[:, 0:NCHUNK], mybir.AxisListType.X, AluOpType.add
            )
            acc_sb = cpool.tile([128, 1], F32, name="acc_sb")
            nc.vector.tensor_copy(acc_sb, psum_acc)
            nc.vector.scalar_tensor_tensor(
                v_sb, s1sum, -float(W), acc_sb,
                AluOpType.mult, AluOpType.add,
            )
            nc.tensor.matmul(psum_tot, v_sb, k2vec, start=True, stop=True)
            nc.vector.tensor_copy(out_sb[0:1, 1:2], psum_tot[0:1, 0:1])

            # ---- small terms: dot/pp/gg off the packed p||g tile ----
            p_ap = pg16[:, 0:64]
            g_ap = pg16[:, 64:128]
            nc.vector.scalar_tensor_tensor(
                junk_stt, p_ap, 1.0, g_ap, AluOpType.mult, AluOpType.mult,
                accum_out=stacked[:, 0:1],
            )
            junk_sq = cpool.tile([128, 64], F32, name="junk_sq")
            nc.scalar.activation(
                junk_sq, p_ap, AF.Square, bias=zeros1,
                accum_out=stacked[:, 1:2],
            )
            nc.scalar.activation(
                junk_sq, g_ap, AF.Square, bias=zeros1,
                accum_out=stacked[:, 2:3],
            )
            nc.tensor.matmul(psum_small, ones32, stacked, start=True, stop=True)

            # ---- scalar chain (partition 0; ts reads PSUM directly;
            # the (1 - .)/8 constant is added on the host) ----
            dot = psum_small[0:1, 0:1]
            pp = psum_small[0:1, 1:2]
            gg = psum_small[0:1, 2:3]
            prodc = sc[0:1, 1:2]
            nc.vector.tensor_scalar(
                prodc, pp, gg, 1e-16, AluOpType.mult, AluOpType.max
            )
            denom = sc[0:1, 7:8]
            nc.scalar.activation(denom, prodc, AF.Sqrt, bias=zeros1[0:1, :])
            inv = sc[0:1, 2:3]
            nc.vector.reciprocal(inv, denom)
            t1 = sc[0:1, 3:4]
            nc.vector.tensor_scalar(
                t1, pp, gg, dot, AluOpType.add, AluOpType.subtract
            )
            t2 = sc[0:1, 4:5]
            nc.vector.tensor_scalar(
                t2, t1, dot, 1.0 / (8.0 * B), AluOpType.subtract, AluOpType.mult
            )
            sims = sc[0:1, 6:7]
            nc.vector.tensor_scalar(
                sims, inv, dot, -1.0 / 8.0, AluOpType.mult, AluOpType.mult
            )
            nc.vector.scalar_tensor_tensor(
                out_sb[0:1, 0:1], sims, 1.0, t2,
                AluOpType.mult, AluOpType.add,
            )
            nc.sync.dma_start(out_d[None, :], out_sb)

    nc.compile()
    return nc


def _pick_w(ps):
    starts = np.arange(0, B, 128)
    for W in W_LADDER:
        s = starts[starts + W < B]
        if s.size == 0 or np.all(ps[s + W] - ps[s + 127] >= MR_BIAS):
            return W
    return B


def kernel(predictions: np.ndarray, correct_output: np.ndarray) -> np.ndarray:
    p = np.ascontiguousarray(np.asarray(predictions, dtype=np.float32))
    g = np.ascontiguousarray(np.asarray(correct_output, dtype=np.float32))

    ps = np.sort(p)
    W = _pick_w(ps)
    if W not in _CACHED:
        _CACHED[W] = _build_nc(W)
    nc = _CACHED[W]

    WIN = 128 * (NCHUNK - 1) + W
    ps16 = np.full(B + WIN, PAD_VAL, dtype=np.float16)
    ps16[:B] = ps.astype(np.float16)
    pg = np.zeros((128, 256), dtype=np.float16)
    pg[:, 0:64] = p.reshape(128, 64)
    pg[:, 64:128] = g.reshape(128, 64)

    in_maps = []
    for k in range(NCORES):
        r0 = k * ROWS_PER_CORE
        in_maps.append(
            {
                "x_win": ps16[r0:r0 + WIN].copy(),
                "p_rows": np.ascontiguousarray(np.concatenate(
                    [ps[r0:r0 + ROWS_PER_CORE].reshape(NCHUNK, 128).T,
                     ps[r0:r0 + ROWS_PER_CORE].reshape(NCHUNK, 128).T
                     + np.float32(MR_BIAS)], axis=1
                )),
                "pg": pg,
            }
        )

    res = None
    last_exc = None
    for _attempt in range(3):
        try:
            res = run_bass_kernel_spmd(nc, in_maps, core_ids=list(range(NCORES)))
            break
        except Exception as e:  # transient NRT/axon device errors
            last_exc = e
            import time as _time
            _time.sleep(1.0)
    if res is None:
        raise last_exc

    # Host gather: per-core scalars + closed-form far-pair constant.
    #   mr = 0.1 - (2/B^2) * (S_near + 0.1*N_far)
    # device out[1] carries -(2/B^2)*S_near_share, out[0] (mse+sim)/8.
    i = np.arange(B, dtype=np.int64)
    hi = np.minimum(128 * (i // 128) + W, B)
    n_near = int(np.sum(hi - i - 1))
    n_far = B * (B - 1) // 2 - n_near
    mr_const = MR_BIAS - 2.0 * MR_BIAS * n_far / (float(B) * float(B))

    total = np.float64(mr_const) + 1.0
    for r in res.results:
        total += np.float64(r["out"][0]) + np.float64(r["out"][1])
    return np.array([total], dtype=np.float32)


if __name__ == "__main__":
    rng = np.random.default_rng(0)
    p = rng.standard_normal(B).astype(np.float32)
    g = rng.standard_normal(B).astype(np.float32)
    print(kernel(p, g))
